# revision 1
# baseline (speedup 1.0000x reference)
"""Trainium2 Bass kernel for Grossberg dynamics (batched gated 17x17 matvecs).

dS/dt = (-DECAY*s + (B-s)*relu(exc) - (C+s)*relu(inh)) / TAU, masked on actions.

Sharding: pure data-parallel over the agent axis across 8 NeuronCores.
Per core: 32768 agents. Macro-tile = 128 partitions x G agents/partition.
Agent a (within a macro block) = p*G + g  (partition-major) so each
partition's HBM reads are contiguous.

Layout per macro-tile:
  wbuf  (128, 2*G*289): [W_pos g=0..G-1 | W_neg g=0..G-1], each row-major 17x17
  s2    (128, 2*G*17):  state duplicated twice (for pos/neg halves)
  prod = wbuf * broadcast(s)   (DVE tensor_tensor, in1 stride-0 on i axis)
  mv   = segmented reduce_add over inner 17 (DVE tensor_reduce axis=X)
  then gates/env/lateral (ACT + GPSIMD small ops), combine, mask, DMA out.
"""

import numpy as np

import concourse.bass as bass
import concourse.bacc as bacc
import concourse.mybir as mybir
from concourse.tile import TileContext
from concourse.bass_utils import run_bass_kernel_spmd

P = 128
N = 17
NN = N * N
NCORES = 8
B_TOTAL = 262144
B_CORE = B_TOTAL // NCORES  # 32768
G = 16                      # agents per partition per macro-tile
MACROS = B_CORE // (P * G)  # 16

FP = mybir.dt.float32
FH = mybir.dt.float16
AX = mybir.AxisListType
OP = mybir.AluOpType
AF = mybir.ActivationFunctionType

# Grossberg constants
TAU, DECAY, B_CAP, C_FLOOR = 0.8, 0.15, 1.0, 0.1
LAT_INHIB, DIV_SIGMA = 3.0, 0.3
ALPHA, BETA = 1.5, 0.75
INV_TAU = 1.0 / TAU                       # 1.25
U_BIAS = DECAY * INV_TAU                  # 0.1875 ; dS = R_e - 0.1*R_i - s*(U_BIAS + R_e + R_i)
LAT_DEN_C = DIV_SIGMA + 1e-6              # 0.300001


def build_program():
    nc = bacc.Bacc()
    st_d = nc.dram_tensor("state", [B_CORE, N], FP, kind="ExternalInput")
    wp_d = nc.dram_tensor("w_pos", [B_CORE, N, N], FH, kind="ExternalInput")
    wn_d = nc.dram_tensor("w_neg", [B_CORE, N, N], FH, kind="ExternalInput")
    fs_d = nc.dram_tensor("feas", [B_CORE, 4], FP, kind="ExternalInput")
    pt_d = nc.dram_tensor("pert", [B_CORE, N], FP, kind="ExternalInput")
    sh_d = nc.dram_tensor("state_h", [B_CORE, N], FH, kind="ExternalInput")
    out_d = nc.dram_tensor("out", [B_CORE, N], FP, kind="ExternalOutput")

    # (MACROS, 128, G*…) views, partition-major agent mapping
    wp_v = wp_d[:, :, :].rearrange("(m p g) i j -> m p (g i j)", p=P, g=G)
    wn_v = wn_d[:, :, :].rearrange("(m p g) i j -> m p (g i j)", p=P, g=G)
    st_v = st_d[:, :].rearrange("(m p g) n -> m p (g n)", p=P, g=G)
    pt_v = pt_d[:, :].rearrange("(m p g) n -> m p (g n)", p=P, g=G)
    sh_v = sh_d[:, :].rearrange("(m p g) n -> m p (g n)", p=P, g=G)
    fs_v = fs_d[:, :].rearrange("(m p g) f -> m p (g f)", p=P, g=G)
    out_v = out_d[:, :].rearrange("(m p g) n -> m p (g n)", p=P, g=G)

    GN = G * N
    with TileContext(nc) as tc:
        with (
            tc.tile_pool(name="big2", bufs=2) as pool2,
            tc.tile_pool(name="big1", bufs=1) as pool1,
        ):
            for m in range(MACROS):
                # ---- loads ----
                wbuf = pool2.tile([P, 2 * G * NN], FH, tag="wbuf")
                nc.sync.dma_start(out=wbuf[:, 0 : G * NN], in_=wp_v[m])
                nc.sync.dma_start(out=wbuf[:, G * NN :], in_=wn_v[m])
                s2 = pool2.tile([P, 2 * GN], FP, tag="s2")
                nc.sync.dma_start(out=s2[:, 0:GN], in_=st_v[m])
                nc.sync.dma_start(out=s2[:, GN:], in_=st_v[m])
                s2h = pool2.tile([P, 2 * GN], FH, tag="s2h")
                nc.sync.dma_start(out=s2h[:, 0:GN], in_=sh_v[m])
                nc.sync.dma_start(out=s2h[:, GN:], in_=sh_v[m])
                pert = pool2.tile([P, GN], FP, tag="pert")
                nc.sync.dma_start(out=pert[:], in_=pt_v[m])
                feas = pool2.tile([P, G * 4], FP, tag="feas")
                nc.sync.dma_start(out=feas[:], in_=fs_v[m])

                # ---- big multiply + segmented reduce (DVE) ----
                prod = pool1.tile([P, 2 * G * NN], FH, tag="prod")
                w4 = wbuf.rearrange("p (k i j) -> p k i j", i=N, j=N)
                p4 = prod.rearrange("p (k i j) -> p k i j", i=N, j=N)
                s4 = s2h.rearrange("p (k j) -> p k j", j=N)[:, :, None, :].broadcast_to(
                    [P, 2 * G, N, N]
                )
                nc.vector.tensor_tensor(out=p4, in0=w4, in1=s4, op=OP.mult)
                mv = pool2.tile([P, 2 * GN], FP, tag="mv")
                nc.vector.tensor_reduce(
                    out=mv[:],
                    in_=prod.rearrange("p (k j) -> p k j", j=N),
                    axis=AX.X,
                    op=OP.add,
                )
                mv3 = mv.rearrange("p (k n) -> p k n", n=N)

                # ---- gates (valence-controlled sigmoid on action rows) ----
                s3 = s2.rearrange("p (k n) -> p k n", n=N)
                pt3 = pert.rearrange("p (g n) -> p g n", n=N)
                ve = pool2.tile([P, 4 * G], FP, tag="ve")
                ve3 = ve.rearrange("p (g f) -> p g f", f=4)
                nc.gpsimd.tensor_tensor(
                    out=ve3, in0=s3[:, 0:G, 13:17], in1=pt3[:, :, 13:17], op=OP.add
                )
                ge = pool2.tile([P, 4 * G], FP, tag="ge")
                nc.scalar.activation(ge[:], ve[:], AF.Sigmoid, scale=ALPHA)
                gi = pool2.tile([P, 4 * G], FP, tag="gi")
                nc.scalar.activation(gi[:], ve[:], AF.Sigmoid, scale=-BETA)
                ge3 = ge.rearrange("p (g f) -> p g f", f=4)
                gi3 = gi.rearrange("p (g f) -> p g f", f=4)
                nc.gpsimd.tensor_tensor(
                    out=mv3[:, 0:G, 9:13], in0=mv3[:, 0:G, 9:13], in1=ge3, op=OP.mult
                )
                nc.gpsimd.tensor_tensor(
                    out=mv3[:, G : 2 * G, 9:13],
                    in0=mv3[:, G : 2 * G, 9:13],
                    in1=gi3,
                    op=OP.mult,
                )

                # ---- environmental drive on the 9 need rows ----
                reluP = pool2.tile([P, GN], FP, tag="reluP")
                nc.scalar.activation(reluP[:], pert[:], AF.Relu)
                reluN = pool2.tile([P, GN], FP, tag="reluN")
                nc.scalar.activation(reluN[:], pert[:], AF.Relu, scale=-1.0)
                rP3 = reluP.rearrange("p (g n) -> p g n", n=N)
                rN3 = reluN.rearrange("p (g n) -> p g n", n=N)
                nc.gpsimd.tensor_tensor(
                    out=mv3[:, 0:G, 0:9], in0=mv3[:, 0:G, 0:9], in1=rP3[:, :, 0:9], op=OP.add
                )
                nc.gpsimd.tensor_tensor(
                    out=mv3[:, G : 2 * G, 0:9],
                    in0=mv3[:, G : 2 * G, 0:9],
                    in1=rN3[:, :, 0:9],
                    op=OP.add,
                )

                # ---- lateral inhibition among the 4 action rows ----
                # all on GPSIMD to avoid cross-engine sync-wait overflow
                a01 = pool2.tile([P, 2 * G], FP, tag="a01")
                a013 = a01.rearrange("p (g f) -> p g f", f=2)
                nc.gpsimd.tensor_tensor(
                    out=a013, in0=s3[:, 0:G, 9:11], in1=s3[:, 0:G, 11:13], op=OP.add
                )
                suma = pool2.tile([P, G], FP, tag="suma")
                nc.gpsimd.tensor_tensor(
                    out=suma[:, :, None],
                    in0=a013[:, :, 0:1],
                    in1=a013[:, :, 1:2],
                    op=OP.add,
                )
                other = pool2.tile([P, 4 * G], FP, tag="other")
                other3 = other.rearrange("p (g f) -> p g f", f=4)
                nc.gpsimd.tensor_tensor(
                    out=other3,
                    in0=suma[:, :, None].broadcast_to([P, G, 4]),
                    in1=s3[:, 0:G, 9:13],
                    op=OP.subtract,
                )
                den = pool2.tile([P, 4 * G], FP, tag="den")
                nc.vector.tensor_scalar_add(out=den[:], in0=other[:], scalar1=LAT_DEN_C)
                recip = pool2.tile([P, 4 * G], FP, tag="recip")
                nc.vector.reciprocal(recip[:], den[:])
                lat = pool2.tile([P, 4 * G], FP, tag="lat")
                nc.vector.scalar_tensor_tensor(
                    out=lat[:],
                    in0=other[:],
                    scalar=LAT_INHIB,
                    in1=recip[:],
                    op0=OP.mult,
                    op1=OP.mult,
                )
                lat3 = lat.rearrange("p (g f) -> p g f", f=4)
                nc.gpsimd.tensor_tensor(
                    out=mv3[:, G : 2 * G, 9:13],
                    in0=mv3[:, G : 2 * G, 9:13],
                    in1=lat3,
                    op=OP.add,
                )

                # ---- shunting combine: dS = R_e - 0.1*R_i - s*(U_BIAS + R_e + R_i)
                # with R = relu(mv * 1.25) (scale folds through relu)
                r = pool2.tile([P, 2 * GN], FP, tag="r")
                nc.scalar.activation(r[:], mv[:], AF.Relu, scale=INV_TAU)
                t1 = pool1.tile([P, GN], FP, tag="t1")
                nc.gpsimd.tensor_tensor(
                    out=t1[:], in0=r[:, 0:GN], in1=r[:, GN:], op=OP.add
                )
                u = pool1.tile([P, GN], FP, tag="u")
                nc.vector.scalar_tensor_tensor(
                    out=u[:],
                    in0=t1[:],
                    scalar=U_BIAS,
                    in1=s2[:, 0:GN],
                    op0=OP.add,
                    op1=OP.mult,
                )
                v = pool1.tile([P, GN], FP, tag="v")
                nc.vector.scalar_tensor_tensor(
                    out=v[:],
                    in0=r[:, GN:],
                    scalar=-C_FLOOR,
                    in1=r[:, 0:GN],
                    op0=OP.mult,
                    op1=OP.add,
                )
                ob = pool2.tile([P, GN], FP, tag="ob")
                nc.gpsimd.tensor_tensor(out=ob[:], in0=v[:], in1=u[:], op=OP.subtract)
                ob3 = ob.rearrange("p (g n) -> p g n", n=N)
                fs3 = feas.rearrange("p (g f) -> p g f", f=4)
                nc.gpsimd.tensor_tensor(
                    out=ob3[:, :, 9:13], in0=ob3[:, :, 9:13], in1=fs3, op=OP.mult
                )

                nc.sync.dma_start(out=out_v[m], in_=ob[:])
    if not nc.is_finalized():
        nc.finalize()
    return nc


def make_in_maps(state, w_pos, w_neg, feasibility, perturbation):
    state = np.ascontiguousarray(np.asarray(state, dtype=np.float32))
    w_pos = np.ascontiguousarray(np.asarray(w_pos, dtype=np.float32))
    w_neg = np.ascontiguousarray(np.asarray(w_neg, dtype=np.float32))
    feas = np.ascontiguousarray(np.asarray(feasibility, dtype=np.float32))
    pert = np.ascontiguousarray(np.asarray(perturbation, dtype=np.float32))
    state_h = state.astype(np.float16)
    w_pos_h = np.ascontiguousarray(w_pos.astype(np.float16))
    w_neg_h = np.ascontiguousarray(w_neg.astype(np.float16))
    in_maps = []
    for c in range(NCORES):
        sl = slice(c * B_CORE, (c + 1) * B_CORE)
        in_maps.append(
            {
                "state": state[sl],
                "state_h": state_h[sl],
                "w_pos": w_pos_h[sl],
                "w_neg": w_neg_h[sl],
                "feas": feas[sl],
                "pert": pert[sl],
            }
        )
    return in_maps


def gather(results):
    return np.concatenate([r["out"] for r in results], axis=0)


def kernel(t=None, state=None, W_pos=None, W_neg=None, feasibility=None, perturbation=None, **_):
    nc = build_program()
    in_maps = make_in_maps(state, W_pos, W_neg, feasibility, perturbation)
    res = run_bass_kernel_spmd(nc, in_maps, list(range(NCORES)))
    return gather(res.results)


if __name__ == "__main__":
    rng = np.random.default_rng(0)
    inputs = {
        "t": rng.standard_normal(1).astype(np.float32),
        "state": rng.random((B_TOTAL, N), dtype=np.float32),
        "W_pos": rng.random((B_TOTAL, N, N), dtype=np.float32),
        "W_neg": rng.random((B_TOTAL, N, N), dtype=np.float32),
        "feasibility": rng.random((B_TOTAL, 4), dtype=np.float32),
        "perturbation": rng.standard_normal((B_TOTAL, N)).astype(np.float32),
    }
    out = kernel(**inputs)
    print(out.shape, out.dtype)

